# revision 1
# baseline (speedup 1.0000x reference)
"""ECE (expected calibration error) kernel for Trainium2, 8 NeuronCores.

Math: per_bin = |avg_conf - avg_acc| * counts/N  ==  |sum_conf - sum_acc| / N
(when counts>0; both sides 0 when counts==0), so

    ECE = (1/(N*C)) * sum_{b,c} | sum_conf[b,c] - sum_acc[b,c] |

The device computes the heavy O(N*C) part per core (data-parallel over N):
  - V[c]    = sum_n conf[n,c]        (softmax column sums, PE-accumulated)
  - s[n]    = sum_c exp(logits[n,c]) (unshifted; logits are bounded, no overflow)
  - m[n]    = max_c logits[n,c]
The host assembles the per-(bin,class) sums from these:
  - bin 0 holds every element with conf <= 1/15; V gives its sum_conf column
    totals directly.  Rows whose max confidence exp(m)/s can exceed 1/15 are
    recomputed exactly on host (a handful of rows) and their >1/15 elements
    are moved from bin 0 into their true bins.
  - sum_acc needs only conf[n, labels[n]] = exp(logits[n,labels[n]]) / s[n].
"""

import os
import sys

import numpy as np

if not os.path.isdir("/opt/trn_rl_repo/concourse"):  # pragma: no cover
    raise RuntimeError("expected /opt/trn_rl_repo with concourse")
if "/opt/trn_rl_repo" not in sys.path:
    sys.path.insert(0, "/opt/trn_rl_repo")

import concourse.bass as bass
import concourse.tile as tile
from concourse import bacc, mybir
from concourse.bass_utils import run_bass_kernel_spmd

N, C, NB = 65536, 1000, 15
N_CORES = 8
N_LOC = N // N_CORES  # 8192
P = 128
T = N_LOC // P  # 64 row-tiles per core
F32 = mybir.dt.float32
BF16 = mybir.dt.bfloat16

_CACHE: dict = {}
LAST_RESULT = None  # BassKernelResults of the most recent run (for profiling)


def _build():
    nc = bacc.Bacc("TRN2", target_bir_lowering=False, debug=False, num_devices=N_CORES)

    logits_ext = nc.declare_dram_parameter("logits", [N_LOC, C], F32, isOutput=False)
    v_ext = nc.declare_dram_parameter("v_out", [1, C], F32, isOutput=True)
    s_ext = nc.declare_dram_parameter("s_out", [P, T], F32, isOutput=True)
    m_ext = nc.declare_dram_parameter("m_out", [P, T], F32, isOutput=True)

    NA = 512  # first PSUM bank width
    NB_ = C - NA  # second

    with tile.TileContext(nc) as tc:
        with (
            tc.tile_pool(name="io", bufs=4) as io_pool,
            tc.tile_pool(name="accum", bufs=1) as acc_pool,
            tc.tile_pool(name="psum", bufs=1, space="PSUM") as psum_pool,
        ):
            s_acc = acc_pool.tile([P, T], F32)
            m_acc = acc_pool.tile([P, T], F32)
            pA = psum_pool.tile([1, NA], F32)
            pB = psum_pool.tile([1, NB_], F32)

            for t in range(T):
                x = io_pool.tile([P, C], F32, tag="x")
                nc.sync.dma_start(out=x[:], in_=logits_ext[t * P : (t + 1) * P, :])

                e = io_pool.tile([P, C], BF16, tag="e")
                nc.scalar.activation(
                    e[:],
                    x[:],
                    mybir.ActivationFunctionType.Exp,
                    accum_out=s_acc[:, t : t + 1],
                )
                nc.vector.tensor_reduce(
                    m_acc[:, t : t + 1],
                    x[:],
                    axis=mybir.AxisListType.X,
                    op=mybir.AluOpType.max,
                )

                w32 = io_pool.tile([P, 1], F32, tag="w32")
                nc.vector.reciprocal(w32[:], s_acc[:, t : t + 1])
                w16 = io_pool.tile([P, 1], BF16, tag="w16")
                nc.vector.tensor_copy(w16[:], w32[:])

                nc.tensor.matmul(
                    pA[:], w16[:], e[:, :NA], start=(t == 0), stop=(t == T - 1)
                )
                nc.tensor.matmul(
                    pB[:], w16[:], e[:, NA:], start=(t == 0), stop=(t == T - 1)
                )

            vout = acc_pool.tile([1, C], F32)
            nc.vector.tensor_copy(vout[:, :NA], pA[:])
            nc.vector.tensor_copy(vout[:, NA:], pB[:])
            nc.sync.dma_start(out=v_ext[:], in_=vout[:])
            nc.sync.dma_start(out=s_ext[:], in_=s_acc[:])
            nc.sync.dma_start(out=m_ext[:], in_=m_acc[:])

    nc.compile()
    return nc


def _get_nc():
    if "nc" not in _CACHE:
        _CACHE["nc"] = _build()
    return _CACHE["nc"]


def kernel(logits: np.ndarray, labels: np.ndarray) -> np.ndarray:
    global LAST_RESULT
    logits = np.ascontiguousarray(logits, dtype=np.float32)
    labels_i = np.asarray(labels).astype(np.int64)

    nc = _get_nc()
    in_maps = [
        {"logits": logits[i * N_LOC : (i + 1) * N_LOC]} for i in range(N_CORES)
    ]
    res = run_bass_kernel_spmd(
        nc,
        in_maps,
        core_ids=list(range(N_CORES)),
        trace=os.environ.get("KERNEL_TRACE", "") == "1",
    )
    LAST_RESULT = res
    outs = res.results

    # --- host reassembly (tiny) ---
    V = np.zeros(C, dtype=np.float64)
    s_glob = np.empty(N, dtype=np.float64)
    m_glob = np.empty(N, dtype=np.float64)
    for i in range(N_CORES):
        V += np.asarray(outs[i]["v_out"]).reshape(C).astype(np.float64)
        # s_out[r, t] -> n_local = t*P + r
        s_glob[i * N_LOC : (i + 1) * N_LOC] = (
            np.asarray(outs[i]["s_out"]).astype(np.float64).T.reshape(N_LOC)
        )
        m_glob[i * N_LOC : (i + 1) * N_LOC] = (
            np.asarray(outs[i]["m_out"]).astype(np.float64).T.reshape(N_LOC)
        )

    sumC = np.zeros((NB, C), dtype=np.float64)
    sumA = np.zeros((NB, C), dtype=np.float64)

    # accuracy side: only conf[n, labels[n]] matters
    lg_label = logits[np.arange(N), labels_i].astype(np.float64)
    conf_label = np.exp(lg_label) / s_glob
    valid = conf_label > 0.0
    bl = np.clip(np.ceil(conf_label * NB).astype(np.int64) - 1, 0, NB - 1)
    np.add.at(sumA, (bl[valid], labels_i[valid]), 1.0)

    # confidence side: everything starts in bin 0 via V; move the rare
    # elements with conf > 1/15 into their true bins (exact host recompute)
    maxconf = np.exp(m_glob) / s_glob
    flagged = np.nonzero(maxconf > (1.0 / NB) * 0.999)[0]
    if flagged.size:
        xr = logits[flagged].astype(np.float64)
        er = np.exp(xr - xr.max(axis=1, keepdims=True))
        cr = er / er.sum(axis=1, keepdims=True)
        rows, cols = np.nonzero(cr > 1.0 / NB)
        if rows.size:
            vals = cr[rows, cols]
            bins = np.clip(np.ceil(vals * NB).astype(np.int64) - 1, 0, NB - 1)
            np.add.at(sumC, (bins, cols), vals)
            np.subtract.at(V, cols, vals)
    sumC[0] += V

    ece = np.abs(sumC - sumA).sum() / (N * C)
    return np.array([ece], dtype=np.float32)
